# revision 12
# baseline (speedup 1.0000x reference)
"""Causal multi-head attention (8 heads, 1x1-conv projections) on 8 TRN2 cores.

Sharding: data-parallel over batch N=8 -> one batch element per NeuronCore.
Per-core kernel (S=1024 pixels, C=E=256 channels, H=8 heads, d=32):
  q = WqT.T @ x, k = WkT.T @ x              (e, s) layout, fp32r matmuls
  vT = x.T @ WvT                            (s, e) layout (transposed v, so the
                                            attention contraction needs no
                                            on-chip transpose of big tensors)
  per head: P^T[sk, sq] = exp(mask(k_h^T q_h))   scores computed TRANSPOSED so
                                            softmax denominator comes from an
                                            appended ones-column in v (M=33)
  out_h = (vAug_h^T @ P^T) -> rows 0..31 numerator^T, row 32 = denominator
  att = num / denom (per-column broadcast via gpsimd partition_broadcast)
  out = WprojT.T @ att + bproj_eff
Host folds: 1/sqrt(d) into Wq/bq; v-bias through the projection
(bproj_eff = bproj + Wproj @ bv, valid because attention rows sum to 1).
P^T and v^T are bf16 (fp32 accumulate); projections and scores are fp32r.

Scheduling notes (vs the first working version):
- Input DMAs are consolidated (packed wq||wk tensor split by m-chunk, packed
  bias vector, host-provided diag-mask tile) and ordered by the critical
  path: wqk[m=0], x[j=0], biases, x[j=1], wqk[m=1], mask, wv, wp.  The ones
  column of vAug and the ones row for the PE broadcast are memset on-chip.
- The diagonal-block causal mask is a bf16 elementwise multiply on DVE
  (mask tile), not a gpsimd affine_select: keeps Pool off the per-chunk
  critical chain.
- The softmax denominator broadcast uses gpsimd partition_broadcast
  (attn ucode library) -- no DRAM bounce, no HWDGE traffic.
- Head 0's first four chunks exp per 512-column window so the ACT exp
  stream starts as soon as the first x half lands.
- The m=0 output projection fires as soon as heads 0..3 are normalized
  (during head 4), and the tail splits head 7's second attn@v half into two
  256-column sub-chains with a DMA-free PE-broadcast normalization, so the
  post-last-exp serial chain is short.
- f32r score matmuls below 256 output columns run at 1/4 rate; the two
  short windows per head are widened to 256 columns (extra columns land in
  PSUM but are never read by exp).
"""

import numpy as np

N_CORES = 8
C = 256      # input channels
E = 256      # embed channels (q/k)
O = 256      # v/out channels
S = 1024     # spatial positions (32*32)
H = 8        # heads
D = 32       # head dim
NCH = 2      # 256 = 2 * 128 partition chunks

_CACHE = {}


def _build_program():
    import concourse.mybir as mybir
    from concourse import bacc
    from concourse import library_config
    from concourse.tile import TileContext

    F32 = mybir.dt.float32
    F32R = mybir.dt.float32r
    BF16 = mybir.dt.bfloat16
    EXP = mybir.ActivationFunctionType.Exp

    nc = bacc.Bacc("TRN2", target_bir_lowering=False, debug=False)

    # fp32r inputs: DMA is an accepted f32r producer, PE rounds on read
    xin = nc.dram_tensor("xin", [C, S], F32R, kind="ExternalInput")
    wqk0 = nc.dram_tensor("wqk0", [C, 2 * 128], F32R, kind="ExternalInput")
    wqk1 = nc.dram_tensor("wqk1", [C, 2 * 128], F32R, kind="ExternalInput")
    wvt = nc.dram_tensor("wvt", [C, O], F32R, kind="ExternalInput")
    wpt = nc.dram_tensor("wpt", [O, O], F32R, kind="ExternalInput")
    biasd = nc.dram_tensor("biasd", [3 * 256], F32, kind="ExternalInput")
    maskd = nc.dram_tensor("maskd", [128, 128], BF16, kind="ExternalInput")
    outd = nc.dram_tensor("out", [O, S], F32, kind="ExternalOutput")

    with TileContext(nc) as tc:
        with (
            tc.tile_pool(name="cst", bufs=1) as cst,
            tc.tile_pool(name="ptp", bufs=3) as ptp,
            tc.tile_pool(name="rbp", bufs=4) as rbp,
            tc.tile_pool(name="osb", bufs=2) as osb,
            tc.tile_pool(name="psc", bufs=2, space="PSUM") as psc,
            tc.tile_pool(name="pav", bufs=4, space="PSUM") as pav,
        ):
            # gpsimd ucode: partition_broadcast lives in the attn library
            nc.gpsimd.load_library(library_config.attn)

            # --- preload exp table + warm the PE clock while DMAs run ---
            dmz = cst.tile([128, 64], F32, tag="dmz")
            nc.vector.memset(dmz, 0.0)
            dme = cst.tile([128, 1], F32, tag="dme")
            nc.scalar.activation(dme, dmz[:, 0:1], EXP)
            wup = cst.tile([128, 64], F32R, tag="wup")
            nc.vector.tensor_copy(wup, dmz)
            pwu = pav.tile([64, 512], F32, tag="pa")
            for _ in range(36):
                nc.tensor.matmul(pwu[:, 0:64], wup, wup[:, 0:64].bitcast(F32R),
                                 start=True, stop=True)

            # on-chip constants (no DMA): vAug ones column + PE-broadcast row
            vaug = cst.tile([128, 8, H, D + 1], BF16, tag="vaug")
            nc.vector.memset(vaug[:, :, :, D], 1.0)
            onc = cst.tile([33, 32], F32, tag="onc")
            nc.vector.memset(onc[32:33, :], 1.0)

            # --- input DMAs ordered along the critical path ---
            wqk = cst.tile([128, NCH, NCH, 2, 128], F32R, tag="wqk")
            wqk_src = [
                d.ap().rearrange("(c p) (t e) -> p c t e", p=128, t=2)
                for d in (wqk0, wqk1)
            ]
            xr = cst.tile([128, NCH, S], F32R, tag="xr")
            xsrc = xin.ap().rearrange("(c p) s -> p c s", p=128)
            bt = cst.tile([128, 3, NCH], F32, tag="bt")
            maskt = cst.tile([128, 128], BF16, tag="maskt")
            wv = cst.tile([128, NCH, 256], F32R, tag="wv")
            wp = cst.tile([128, NCH, 256], F32R, tag="wp")

            nc.sync.dma_start(out=wqk[:, 0], in_=wqk_src[0])
            nc.sync.dma_start(out=xr[:, :, 0:512], in_=xsrc[:, :, 0:512])
            nc.sync.dma_start(
                out=bt, in_=biasd.ap().rearrange("(b m p) -> p b m", p=128, b=3)
            )
            nc.sync.dma_start(out=xr[:, :, 512:1024], in_=xsrc[:, :, 512:1024])
            nc.sync.dma_start(out=wqk[:, 1], in_=wqk_src[1])
            nc.sync.dma_start(out=maskt, in_=maskd.ap())
            nc.sync.dma_start(out=wv, in_=wvt.ap().rearrange("(c p) e -> p c e", p=128))
            nc.sync.dma_start(out=wp, in_=wpt.ap().rearrange("(c p) e -> p c e", p=128))

            q_sb = cst.tile([128, NCH, S], F32R, tag="q_sb")
            k_sb = cst.tile([128, NCH, S], F32R, tag="k_sb")
            att = cst.tile([128, NCH, S], F32R, tag="att")

            def qk_proj_unit(t, m, j, pool=None):
                # t: 0 = q, 1 = k
                dst = (q_sb, k_sb)[t]
                pp = (pool or psc).tile([128, 512], F32, tag="sc" if pool is None else "pa")
                for c in range(2):
                    nc.tensor.matmul(
                        pp,
                        wqk[:, m, c, t, :],
                        xr[:, c, j * 512:(j + 1) * 512],
                        start=(c == 0), stop=(c == 1),
                    )
                nc.vector.tensor_scalar_add(
                    dst[:, m, j * 512:(j + 1) * 512], pp, bt[:, t, m:m + 1]
                )

            def v_proj_unit(i):
                pv = psc.tile([128, 512], F32, tag="sc")
                for c in range(2):
                    nc.tensor.matmul(
                        pv[:, 0:256],
                        xr[:, c, i * 128:(i + 1) * 128],
                        wv[:, c, :],
                        start=(c == 0), stop=(c == 1),
                    )
                nc.vector.tensor_copy(
                    vaug[:, i, :, 0:D],
                    pv[:, 0:256].rearrange("p (h d) -> p h d", h=H),
                )

            def mask_mult(pts, i):
                # zero the strictly-lower part of the diagonal block
                nc.vector.tensor_mul(
                    pts[:, i, 128 * i:128 * (i + 1)],
                    pts[:, i, 128 * i:128 * (i + 1)],
                    maskt,
                )

            def scores_win(h, ps, pts, i, j):
                # matmul + exp for sq window j (512 cols) of sk chunk i
                m, r = h // 4, h % 4
                rows = slice(32 * r, 32 * r + 32)
                we = 512 * (j + 1)
                if we <= 128 * i:
                    return
                ws = max(512 * j, 128 * i)
                # f32r matmuls under 256 output columns run at 1/4 rate:
                # widen (exp never reads the extra columns)
                ws_mm = max(min(ws, we - 256), 512 * j)
                nc.tensor.matmul(
                    ps[:, ws_mm:we],
                    k_sb[rows, m, 128 * i:128 * (i + 1)],
                    q_sb[rows, m, ws_mm:we],
                    start=True, stop=True,
                    tile_position=(32 * r, 0),
                )
                nc.scalar.activation(pts[:, i, ws:we], ps[:, ws:we], EXP)
                if ws <= 128 * i:
                    mask_mult(pts, i)

            def scores_chunk(h, pts, i):
                # whole-chunk variant: window matmuls, then one exp.  Chunks
                # >= 4 only touch columns 512:1024, so they take a 1-bank
                # psum slot from pav -- keeps the 2-slot psc ring free of
                # head-boundary write-after-read stalls.
                m, r = h // 4, h % 4
                rows = slice(32 * r, 32 * r + 32)
                if i < 4:
                    ps = psc.tile([128, S], F32, tag="sc")
                    off = 0
                else:
                    ps = pav.tile([128, 512], F32, tag="pa")
                    off = 512
                for j in range(2):
                    we = 512 * (j + 1)
                    if we <= 128 * i:
                        continue
                    ws = max(512 * j, 128 * i)
                    ws_mm = max(min(ws, we - 256), 512 * j)
                    nc.tensor.matmul(
                        ps[:, ws_mm - off:we - off],
                        k_sb[rows, m, 128 * i:128 * (i + 1)],
                        q_sb[rows, m, ws_mm:we],
                        start=True, stop=True,
                        tile_position=(32 * r, 0),
                    )
                nc.scalar.activation(pts[:, i, 128 * i:S], ps[:, 128 * i - off:S - off], EXP)
                mask_mult(pts, i)

            def attnv(h, pts, j, fast=False):
                # attn@v for sq-half j; own 1-bank psum slot so the slow
                # normalization chain never blocks the scores/exp pipeline
                m, r = h // 4, h % 4
                pa = pav.tile([33, 512], F32, tag="pa")
                ii = [i for i in range(8) if 128 * i < 512 * (j + 1)]
                for idx, i in enumerate(ii):
                    ws = max(512 * j, 128 * i)
                    we = 512 * (j + 1)
                    nc.tensor.matmul(
                        pa[:, ws - 512 * j:we - 512 * j],
                        vaug[:, i, h, :],
                        pts[:, i, ws:we],
                        start=(idx == 0), stop=(idx == len(ii) - 1),
                    )
                if fast:
                    # tail chain: recip -> PE outer-product broadcast -> DVE
                    # copy to SBUF -> mul (no gpsimd latency)
                    rf = rbp.tile([33, 512], F32R, tag="rff")
                    with nc.allow_low_precision(reason="softmax recip in f32r"):
                        nc.vector.reciprocal(rf, pa)
                    pb = pav.tile([32, 512], F32, tag="pa")
                    nc.tensor.matmul(pb, onc[32:33, :].bitcast(F32R), rf[32:33, :],
                                     start=True, stop=True)
                    rb = rbp.tile([32, 512], F32, tag="rb")
                    nc.vector.tensor_copy(rb, pb)
                else:
                    rf = rbp.tile([1, 512], F32, tag="rf")
                    nc.vector.reciprocal(rf, pa[32:33, :])
                    rb = rbp.tile([32, 512], F32, tag="rb")
                    nc.gpsimd.partition_broadcast(rb, rf)
                nc.vector.tensor_mul(
                    att[32 * r:32 * r + 32, m, 512 * j:512 * (j + 1)],
                    pa[0:32, :], rb,
                )

            def attnv_tail_mms(h, pts, q0, q1, pa, ii, first, last_mm):
                # partial attn@v accumulation for sq columns [q0, q1)
                for idx, i in enumerate(ii):
                    ws = max(q0, 128 * i)
                    nc.tensor.matmul(
                        pa[:, ws - q0:q1 - q0],
                        vaug[:, i, h, :],
                        pts[:, i, ws:q1],
                        start=(first and idx == 0),
                        stop=(last_mm and idx == len(ii) - 1),
                    )

            def attnv_tail_norm(h, pa, q0, q1):
                # PE-broadcast normalization (DMA- and Pool-free tail chain)
                m, r = h // 4, h % 4
                w = q1 - q0
                rf = rbp.tile([33, 512], F32R, tag="rff")
                with nc.allow_low_precision(reason="softmax recip in f32r"):
                    nc.vector.reciprocal(rf[:, 0:w], pa[:, 0:w])
                pb = pav.tile([32, 512], F32, tag="pa")
                nc.tensor.matmul(pb[:, 0:w], onc[32:33, :].bitcast(F32R), rf[32:33, 0:w],
                                 start=True, stop=True)
                rb = rbp.tile([32, 512], F32, tag="rb")
                nc.scalar.copy(rb[:, 0:w], pb[:, 0:w])
                nc.vector.tensor_mul(
                    att[32 * r:32 * r + 32, m, q0:q1],
                    pa[0:32, 0:w], rb[:, 0:w],
                )

            out_ap = outd.ap().rearrange("(m p) s -> p m s", p=128)

            def outproj_unit(m, q0, q1, ot2=None):
                # output projection + bias for out-chunk m, columns [q0, q1)
                w = q1 - q0
                po = pav.tile([128, 512], F32, tag="pa")
                for c in range(2):
                    nc.tensor.matmul(
                        po[:, 0:w],
                        wp[:, c, m * 128:(m + 1) * 128],
                        att[:, c, q0:q1],
                        start=(c == 0), stop=(c == 1),
                    )
                if ot2 is not None:
                    nc.scalar.add(ot2[:, m, 0:w], po[:, 0:w], bt[:, 2, m:m + 1])
                    return
                ot = osb.tile([128, 512], F32, tag="ot")
                nc.scalar.add(ot[:, 0:w], po[:, 0:w], bt[:, 2, m:m + 1])
                nc.sync.dma_start(out=out_ap[:, m, q0:q1], in_=ot[:, 0:w])

            def outproj_pair(q0, q1):
                # both m chunks into one shared tile -> single output DMA
                w = q1 - q0
                ot2 = osb.tile([128, 2, 512], F32, tag="ot2")
                outproj_unit(0, q0, q1, ot2=ot2)
                outproj_unit(1, q0, q1, ot2=ot2)
                nc.sync.dma_start(out=out_ap[:, :, q0:q1], in_=ot2[:, :, 0:w])

            # --- emission schedule ---
            pts_tiles = {}

            def get_pts(h):
                if h not in pts_tiles:
                    pts = ptp.tile([128, 8, S], BF16, tag="pts")
                    pts_tiles[h] = pts
                return pts_tiles[h]

            # head 0 primes the ACT stream: j=0 windows of chunks 0..3 only
            # need the first x half.  Two [128, S] psum slots ping-pong
            # across the four chunks; the j=1 q/k projections borrow pav
            # psum so they don't alias the in-flight score windows.
            qk_proj_unit(0, 0, 0)
            qk_proj_unit(1, 0, 0)
            pts0 = get_pts(0)
            slot_a = psc.tile([128, S], F32, tag="sc")
            slot_b = psc.tile([128, S], F32, tag="sc")
            ps0 = {0: slot_a, 1: slot_b, 2: slot_a, 3: slot_b}
            for i in range(4):
                scores_win(0, ps0[i], pts0, i, 0)
            qk_proj_unit(0, 0, 1, pool=pav)
            qk_proj_unit(1, 0, 1, pool=pav)
            for i in range(4):
                scores_win(0, ps0[i], pts0, i, 1)

            def sc(h, i):
                scores_chunk(h, get_pts(h), i)

            for h in range(H):
                last = h == H - 1
                if h > 0:
                    attnv(h - 1, pts_tiles[h - 1], 0)
                for i in range(1, 8):
                    if h == 0 and i < 4:
                        pass  # chunks 0..3 already emitted above
                    else:
                        if i == 3 and h > 0:
                            attnv(h - 1, pts_tiles[h - 1], 1)
                            pts_tiles.pop(h - 1)
                        if i == 4 and last:
                            attnv(h, pts_tiles[h], 0)
                        if i == 7 and not last:
                            sc(h + 1, 0)
                        sc(h, i)
                    if h == 0 and 1 <= i <= 4:
                        v_proj_unit(2 * (i - 1))
                        v_proj_unit(2 * (i - 1) + 1)
                    if h == 1 and 1 <= i <= 4:
                        qk_proj_unit((i + 1) % 2, 1, (i - 1) // 2)
                # keep the exp stream fed across the head boundary: emit
                # nothing else here; attnv of this head is deferred
            # tail: head 7.  attnv(7, 0) was emitted at i == 5, so att's
            # j=0 half completes for all heads while the last exps retire;
            # the j=1 half is split into two 256-column sub-chains, with the
            # [768:1024) accumulation pre-issued for chunks 0..6 so only one
            # 128-column matmul plus a short PE-broadcast normalization and
            # a 256-column projection remain after the final exp.
            p7 = pts_tiles[H - 1]
            pa_a = pav.tile([33, 512], F32, tag="pa")
            attnv_tail_mms(H - 1, p7, 512, 768, pa_a,
                           list(range(6)), True, True)
            outproj_unit(0, 0, 512)
            outproj_unit(1, 0, 512)
            attnv_tail_norm(H - 1, pa_a, 512, 768)
            pa_b = pav.tile([33, 512], F32, tag="pa")
            attnv_tail_mms(H - 1, p7, 768, 1024, pa_b,
                           list(range(7)), True, False)
            attnv_tail_mms(H - 1, p7, 768, 1024, pa_b, [7], False, True)
            attnv_tail_norm(H - 1, pa_b, 768, 1024)
            outproj_pair(512, 768)
            outproj_pair(768, 1024)

    nc.compile()
    return nc


def get_program():
    if "nc" not in _CACHE:
        _CACHE["nc"] = _build_program()
    return _CACHE["nc"]


def kernel(x, wq, bq, wkv, bkv, wproj, bproj):
    import ml_dtypes
    from concourse.bass_utils import run_bass_kernel_spmd

    nc = get_program()

    x = np.asarray(x, dtype=np.float32)
    n = x.shape[0]
    assert n == N_CORES and x.shape[1:] == (C, 32, 32)

    scale = 1.0 / np.sqrt(np.float32(D))
    wq_s = np.asarray(wq, np.float32) * scale
    bq_s = np.asarray(bq, np.float32) * scale
    wk = np.asarray(wkv[:E], np.float32)
    bk = np.asarray(bkv[:E], np.float32)
    wv = np.asarray(wkv[E:], np.float32)
    bv = np.asarray(bkv[E:], np.float32)
    wproj = np.asarray(wproj, np.float32)
    bproj_eff = (np.asarray(bproj, np.float32)
                 + wproj.astype(np.float64) @ bv.astype(np.float64)).astype(np.float32)

    # mask[sk, sq] keeps sq >= sk (upper triangle incl. diagonal of the
    # transposed scores' diagonal block)
    mask = np.triu(np.ones((128, 128), np.float32)).astype(ml_dtypes.bfloat16)

    shared = {
        "wqk0": np.ascontiguousarray(
            np.concatenate([wq_s.T[:, 0:128], wk.T[:, 0:128]], axis=1)),
        "wqk1": np.ascontiguousarray(
            np.concatenate([wq_s.T[:, 128:256], wk.T[:, 128:256]], axis=1)),
        "wvt": np.ascontiguousarray(wv.T),
        "wpt": np.ascontiguousarray(wproj.T),
        "biasd": np.ascontiguousarray(
            np.concatenate([bq_s, bk, bproj_eff])),
        "maskd": mask,
    }
    in_maps = [
        {"xin": np.ascontiguousarray(x[i].reshape(C, S)), **shared}
        for i in range(N_CORES)
    ]
    res = run_bass_kernel_spmd(nc, in_maps, core_ids=list(range(N_CORES)))
    out = np.stack([res.results[i]["out"].reshape(O, 32, 32) for i in range(N_CORES)])
    return out.astype(np.float32)
